# revision 7
# baseline (speedup 1.0000x reference)
"""Trainium2 Bass kernel for ClusterBasedSpatialAwarenessModule.

out[s,b,:] = x[s,b,:] + sum_m sum_{|o|<=r_m, 0<=s+o<S}
    sigmoid(a_m * cos(xnorm[s+o], winmean_m[s]) + b_m) * (x[s+o] @ W_m^T + bias_m)

Sharding: data-parallel over batch B=8 across the 8 cores (one sample each);
W/b/alphas/betas replicated.  Layout inside a core is "T-major": feature dim D
on partitions (4 chunks of 128), sequence dim S on the free axis, so windowed
ops are free-axis shifts and the cosine gram is a plain banded matmul.
"""

import sys

import numpy as np

_TRN_REPO = "/opt/trn_rl_repo"
if _TRN_REPO not in sys.path:
    sys.path.insert(0, _TRN_REPO)

import ml_dtypes  # noqa: E402

import concourse.bacc as bacc  # noqa: E402
import concourse.mybir as mybir  # noqa: E402
import concourse.tile as tile  # noqa: E402
from concourse.bass_utils import run_bass_kernel_spmd  # noqa: E402

S, B, D = 1024, 8, 512
NB = S // 128          # 8 sequence blocks
DC = D // 128          # 4 feature chunks
RADII = (3, 5, 7)
NL = len(RADII)
WCOL = 142             # gram window columns: 128 + 2*7
A7W = 8 + S + 7        # padded cumsum row length
XNW = 7 + S + 7        # padded xnorm row length

F32 = mybir.dt.float32
F32R = mybir.dt.float32r
BF16 = mybir.dt.bfloat16
AF = mybir.ActivationFunctionType
OP = mybir.AluOpType

_CACHE = {}


def _host_masks() -> np.ndarray:
    """[3 levels x 3 blocktypes, 128, WCOL] band+edge validity masks."""
    m = np.zeros((NL * 3, 128, WCOL), np.float32)
    sl = np.arange(128)[:, None]
    c = np.arange(WCOL)[None, :]
    for mi, r in enumerate(RADII):
        for ti, s0 in enumerate((0, 128, S - 128)):
            src = s0 + c - 7
            band = np.abs(c - 7 - sl) <= r
            valid = (src >= 0) & (src < S)
            m[mi * 3 + ti] = (band & valid).astype(np.float32)
    return m


def _build():
    nc = bacc.Bacc(
        "TRN2",
        target_bir_lowering=False,
        debug=False,
        num_devices=1,
    )
    x_d = nc.dram_tensor("x0", [S, D], F32, kind="ExternalInput")
    w_d = nc.dram_tensor("w0", [NL, D, D], F32, kind="ExternalInput")
    b_d = nc.dram_tensor("b0", [1, NL * D], F32, kind="ExternalInput")
    ab_d = nc.dram_tensor("ab0", [1, 2 * NL], F32, kind="ExternalInput")
    mask_d = nc.dram_tensor("mask0", [NL * 3, 128, WCOL], BF16, kind="ExternalInput")
    iden_d = nc.dram_tensor("iden0", [128, 128], F32, kind="ExternalInput")
    ones_d = nc.dram_tensor("ones0", [128, 1], BF16, kind="ExternalInput")
    out_d = nc.dram_tensor("out0", [S, D], F32, kind="ExternalOutput")

    with tile.TileContext(nc) as tc:
        _emit(nc, tc, x_d, w_d, b_d, ab_d, mask_d, iden_d, ones_d, out_d)
    nc.compile()
    return nc


def _emit(nc, tc, x_d, w_d, b_d, ab_d, mask_d, iden_d, ones_d, out_d):
    import contextlib

    ctx = contextlib.ExitStack()
    with ctx:
        P = ctx.enter_context(tc.tile_pool(name="persist", bufs=1))

        x_nat = P.tile([128, NB * 512], F32, tag="x_nat")
        xT = P.tile([128, DC * 1024], F32R, tag="xT")
        xnT = P.tile([128, DC * XNW], BF16, tag="xnT")
        a7 = P.tile([128, DC * A7W], F32, tag="a7")
        wsT = P.tile([128, DC * 1024], BF16, tag="wsT")
        tfm = P.tile([128, NL * 3 * 512], F32R, tag="tfm")      # ring over blocks
        wtsT = P.tile([128, NL * NB * 384], F32R, tag="wtsT")
        wT = P.tile([128, NL * DC * 512], F32R, tag="wT")
        brep = P.tile([128, NL * 512], F32, tag="brep")
        maskt = P.tile([128, NL * 3 * WCOL], BF16, tag="maskt")
        rrep = P.tile([128, 1024], F32, tag="rrep")
        iden = P.tile([128, 128], F32, tag="iden")
        onesc = P.tile([128, 1], BF16, tag="onesc")
        nrm_x = P.tile([1, 1024], F32, tag="nrm_x")
        rn_x = P.tile([1, 1024], F32, tag="rn_x")
        nrm_ws = P.tile([1, 1024], F32, tag="nrm_ws")
        rn_cols = P.tile([128, NL * NB], F32, tag="rn_cols")
        ab_row = P.tile([1, 2 * NL], F32, tag="ab_row")
        beta_cols = P.tile([128, NL], F32, tag="beta_cols")
        a2inv = P.tile([1, NL], F32, tag="a2inv")
        asq = P.tile([1, NL], F32, tag="asq")

        # ---------------- Phase 0: loads ----------------
        nc.sync.dma_start(
            x_nat[:].rearrange("p (bk d) -> p bk d", bk=NB),
            x_d.ap().rearrange("(bk p) d -> p bk d", p=128),
        )
        nc.sync.dma_start(iden[:], iden_d.ap())
        nc.sync.dma_start(onesc[:], ones_d.ap())
        nc.sync.dma_start(
            maskt[:].rearrange("p (m c) -> p m c", m=NL * 3),
            mask_d.ap().rearrange("m p c -> p m c"),
        )
        nc.sync.dma_start(ab_row[:], ab_d.ap())
        brow = P.tile([1, NL * 512], F32, tag="brow")
        nc.sync.dma_start(brow[:], b_d.ap())
        nc.gpsimd.partition_broadcast(brep[:], brow[:])
        nc.gpsimd.partition_broadcast(beta_cols[:], ab_row[0:1, NL : 2 * NL])
        # per-level 1/alpha^2 (assumes alpha > 0, true for torch.ones init)
        nc.scalar.activation(asq[:], ab_row[0:1, 0:NL], AF.Square)
        nc.vector.reciprocal(a2inv[:], asq[:])

        # ---------------- Phase 1+2: W / x transposes ----------------
        wnat_pool = ctx.enter_context(tc.tile_pool(name="wnat", bufs=2))
        with tc.tile_pool(name="pst", bufs=3, space="PSUM") as pst:
            for m in range(NL):
                wnat = wnat_pool.tile([128, DC * 512], F32, tag="wnat")
                nc.sync.dma_start(
                    wnat[:].rearrange("p (qc din) -> p qc din", qc=DC),
                    w_d.ap()[m].rearrange("(qc p) din -> p qc din", p=128),
                )
                for kc in range(DC):
                    ps = pst.tile([128, 512], F32, tag="pst")
                    for qc in range(DC):
                        nc.tensor.transpose(
                            ps[:, qc * 128 : (qc + 1) * 128],
                            wnat[:, qc * 512 + kc * 128 : qc * 512 + (kc + 1) * 128],
                            iden[:],
                        )
                    eng = nc.scalar if kc % 2 == 0 else nc.vector
                    if kc % 2 == 0:
                        nc.scalar.copy(wT[:, (m * DC + kc) * 512 : (m * DC + kc + 1) * 512], ps[:])
                    else:
                        nc.vector.tensor_copy(wT[:, (m * DC + kc) * 512 : (m * DC + kc + 1) * 512], ps[:])
            for bk in range(NB):
                ps = pst.tile([128, 512], F32, tag="pst")
                for dc in range(DC):
                    nc.tensor.transpose(
                        ps[:, dc * 128 : (dc + 1) * 128],
                        x_nat[:, bk * 512 + dc * 128 : bk * 512 + (dc + 1) * 128],
                        iden[:],
                    )
                dst = xT[:].rearrange("p (dc s) -> p dc s", dc=DC)[
                    :, :, bk * 128 : (bk + 1) * 128
                ]
                if bk % 2 == 0:
                    nc.scalar.copy(dst, ps[:].rearrange("p (dc s) -> p dc s", dc=DC))
                else:
                    nc.vector.tensor_copy(dst, ps[:].rearrange("p (dc s) -> p dc s", dc=DC))

        # zero pads of xnT / A7
        for dc in range(DC):
            nc.vector.memset(xnT[:, dc * XNW : dc * XNW + 7], 0)
            nc.vector.memset(xnT[:, dc * XNW + 7 + S : (dc + 1) * XNW], 0)
            nc.vector.memset(a7[:, dc * A7W : dc * A7W + 8], 0)

        sq_pool = ctx.enter_context(tc.tile_pool(name="sq", bufs=2))

        with (
            tc.tile_pool(name="psrow", bufs=2, space="PSUM") as psrow,
            tc.tile_pool(name="pscol", bufs=2, space="PSUM") as pscol,
            tc.tile_pool(name="psg", bufs=2, space="PSUM") as psg,
            tc.tile_pool(name="pspk", bufs=2, space="PSUM") as pspk,
        ):
            # ---------------- Phase 3: x norms -> xnT ----------------
            rows = [psrow.tile([1, 512], F32, tag="psrow", name=f"psrow{jj}") for jj in range(2)]
            for dc in range(DC):
                sq = sq_pool.tile([128, 1024], BF16, tag="sq")
                eng = nc.scalar if dc < 2 else nc.vector
                if eng is nc.scalar:
                    nc.scalar.activation(sq[:], xT[:, dc * 1024 : (dc + 1) * 1024].bitcast(F32), AF.Square)
                else:
                    nc.vector.tensor_mul(
                        sq[:],
                        xT[:, dc * 1024 : (dc + 1) * 1024].bitcast(F32),
                        xT[:, dc * 1024 : (dc + 1) * 1024].bitcast(F32),
                    )
                for jc in range(2):
                    nc.tensor.matmul(
                        rows[jc][:],
                        onesc[:],
                        sq[:, jc * 512 : (jc + 1) * 512],
                        start=(dc == 0),
                        stop=(dc == DC - 1),
                    )
            for jc in range(2):
                nc.scalar.activation(
                    nrm_x[0:1, jc * 512 : (jc + 1) * 512], rows[jc][:], AF.Sqrt
                )
            nc.vector.reciprocal(rn_x[:], nrm_x[:])
            nc.gpsimd.partition_broadcast(rrep[:], rn_x[:])
            for dc in range(DC):
                nc.vector.tensor_mul(
                    xnT[:, dc * XNW + 7 : dc * XNW + 7 + S],
                    xT[:, dc * 1024 : (dc + 1) * 1024].bitcast(F32),
                    rrep[:],
                )

            # ---------------- Phase 4: cumsum ----------------
            for dc in range(DC):
                nc.vector.tensor_tensor_scan(
                    a7[:, dc * A7W + 8 : dc * A7W + 8 + S],
                    xT[:, dc * 1024 : (dc + 1) * 1024].bitcast(F32),
                    xT[:, dc * 1024 : (dc + 1) * 1024].bitcast(F32),
                    0.0,
                    OP.add,
                    OP.bypass,
                )
                nc.vector.tensor_scalar_add(
                    a7[:, dc * A7W + 8 + S : (dc + 1) * A7W],
                    a7[:, dc * A7W : dc * A7W + 7],
                    a7[:, dc * A7W + 7 + S : dc * A7W + 8 + S],
                )

            # ---------------- Phase 5: per-level wts ----------------
            for m, r in enumerate(RADII):
                # window sums via cumsum difference
                for dc in range(DC):
                    eng = nc.gpsimd if m == 2 else nc.vector
                    eng.tensor_sub(
                        wsT[:, dc * 1024 : (dc + 1) * 1024],
                        a7[:, dc * A7W + r + 8 : dc * A7W + r + 8 + S],
                        a7[:, dc * A7W + 7 - r : dc * A7W + 7 - r + S],
                    )
                # ||ws|| rows (scaled by 1/alpha)
                rows = [psrow.tile([1, 512], F32, tag="psrow", name=f"psrow{jj}") for jj in range(2)]
                for dc in range(DC):
                    sq = sq_pool.tile([128, 1024], BF16, tag="sq")
                    if m == 0:
                        nc.scalar.activation(
                            sq[:], wsT[:, dc * 1024 : (dc + 1) * 1024], AF.Square
                        )
                    else:
                        nc.vector.tensor_mul(
                            sq[:],
                            wsT[:, dc * 1024 : (dc + 1) * 1024],
                            wsT[:, dc * 1024 : (dc + 1) * 1024],
                        )
                    for jc in range(2):
                        nc.tensor.matmul(
                            rows[jc][:],
                            onesc[:],
                            sq[:, jc * 512 : (jc + 1) * 512],
                            start=(dc == 0),
                            stop=(dc == DC - 1),
                        )
                for jc in range(2):
                    nc.scalar.activation(
                        nrm_ws[0:1, jc * 512 : (jc + 1) * 512],
                        rows[jc][:],
                        AF.Sqrt,
                        scale=a2inv[0:1, m : m + 1],
                    )
                # row -> per-block columns; sigmoid scale = alpha/||ws||
                for bk in range(NB):
                    pc = pscol.tile([128, 1], F32, tag="pscol")
                    nc.tensor.transpose(
                        pc[:], nrm_ws[0:1, bk * 128 : (bk + 1) * 128], iden[0:1, 0:1]
                    )
                    nc.vector.reciprocal(rn_cols[:, m * NB + bk : m * NB + bk + 1], pc[:])

                for bk in range(NB):
                    pg = psg.tile([128, WCOL], F32, tag="psg")
                    for dc in range(DC):
                        nc.tensor.matmul(
                            pg[:],
                            wsT[:, dc * 1024 + bk * 128 : dc * 1024 + (bk + 1) * 128],
                            xnT[:, dc * XNW + bk * 128 : dc * XNW + bk * 128 + WCOL],
                            start=(dc == 0),
                            stop=(dc == DC - 1),
                        )
                    wts = sq_pool.tile([128, WCOL], F32, tag="wts")
                    nc.scalar.activation(
                        wts[:],
                        pg[:],
                        AF.Sigmoid,
                        bias=beta_cols[:, m : m + 1],
                        scale=rn_cols[:, m * NB + bk : m * NB + bk + 1],
                    )
                    mtype = 0 if bk == 0 else (2 if bk == NB - 1 else 1)
                    nc.gpsimd.tensor_mul(
                        wts[:], wts[:], maskt[:, (m * 3 + mtype) * WCOL : (m * 3 + mtype + 1) * WCOL]
                    )
                    pk = pspk.tile([128, 384], F32, tag="pspk")
                    nc.tensor.transpose(
                        pk[:, 0:128],
                        wts[:, 7:135],
                        iden[:],
                    )
                    nc.tensor.transpose(
                        pk[0:7, 128:256],
                        wts[:, 0:7],
                        iden[:],
                    )
                    nc.tensor.transpose(
                        pk[0:7, 256:384],
                        wts[:, 135:142],
                        iden[:],
                    )
                    dst = wtsT[:, (m * NB + bk) * 384 : (m * NB + bk + 1) * 384]
                    if (m * NB + bk) % 2 == 0:
                        nc.vector.tensor_copy(dst, pk[:])
                    else:
                        nc.scalar.copy(dst, pk[:])

        # ---------------- Phase 6: tfm ring + accumulation ----------------
        with (
            tc.tile_pool(name="pstf", bufs=2, space="PSUM") as pstf,
            tc.tile_pool(name="psout", bufs=2, space="PSUM") as psout,
            tc.tile_pool(name="tfmh", bufs=3) as tfmh,
        ):

            def emit_tfm(m, j):
                ps = pstf.tile([128, 512], F32, tag="pstf")
                for kc in range(DC):
                    nc.tensor.matmul(
                        ps[:],
                        xT[:, kc * 1024 + j * 128 : kc * 1024 + (j + 1) * 128],
                        wT[:, (m * DC + kc) * 512 : (m * DC + kc + 1) * 512],
                        start=(kc == 0),
                        stop=(kc == DC - 1),
                    )
                slot = (m * 3 + j % 3) * 512
                nc.vector.tensor_add(
                    tfm[:, slot : slot + 512], ps[:], brep[:, m * 512 : (m + 1) * 512]
                )

            for bk in range(NB):
                for m in range(NL):
                    if bk == 0:
                        emit_tfm(m, 0)
                    if bk < NB - 1:
                        emit_tfm(m, bk + 1)
                po = psout.tile([128, 512], F32, tag="psout")
                n_mm = NL * (1 + (bk > 0) + (bk < NB - 1))
                k = 0
                for m in range(NL):
                    base = (m * NB + bk) * 384
                    if bk > 0:
                        slot = (m * 3 + (bk - 1) % 3) * 512
                        th = tfmh.tile([7, 512], F32R, tag="tfmh")
                        nc.sync.dma_start(th[:], tfm[121:128, slot : slot + 512])
                        nc.tensor.matmul(
                            po[:],
                            wtsT[0:7, base + 128 : base + 256],
                            th[:],
                            start=(k == 0),
                            stop=(k == n_mm - 1),
                        )
                        k += 1
                    slot = (m * 3 + bk % 3) * 512
                    nc.tensor.matmul(
                        po[:],
                        wtsT[0:128, base : base + 128],
                        tfm[0:128, slot : slot + 512],
                        start=(k == 0),
                        stop=(k == n_mm - 1),
                    )
                    k += 1
                    if bk < NB - 1:
                        slot = (m * 3 + (bk + 1) % 3) * 512
                        nc.tensor.matmul(
                            po[:],
                            wtsT[0:7, base + 256 : base + 384],
                            tfm[0:7, slot : slot + 512],
                            start=(k == 0),
                            stop=(k == n_mm - 1),
                        )
                        k += 1
                nc.vector.tensor_add(
                    x_nat[:, bk * 512 : (bk + 1) * 512],
                    po[:],
                    x_nat[:, bk * 512 : (bk + 1) * 512],
                )
                nc.sync.dma_start(
                    out_d.ap().rearrange("(bk p) d -> p bk d", p=128)[:, bk, :],
                    x_nat[:, bk * 512 : (bk + 1) * 512],
                )


def _get_program():
    if "nc" not in _CACHE:
        _CACHE["nc"] = _build()
    return _CACHE["nc"]


def kernel(x, W, b, alphas, betas):
    x = np.asarray(x, np.float32)
    W = np.asarray(W, np.float32)
    b = np.asarray(b, np.float32)
    ab = np.stack([np.asarray(alphas, np.float32), np.asarray(betas, np.float32)])
    shared = {
        "w0": np.ascontiguousarray(W),
        "b0": np.ascontiguousarray(b.reshape(1, NL * D)),
        "ab0": np.ascontiguousarray(ab.reshape(1, 2 * NL)),
        "mask0": _host_masks().astype(ml_dtypes.bfloat16),
        "iden0": np.eye(128, dtype=np.float32),
        "ones0": np.ones((128, 1), ml_dtypes.bfloat16),
    }
    in_maps = [
        {**shared, "x0": np.ascontiguousarray(x[:, c, :])} for c in range(B)
    ]
    nc = _get_program()
    res = run_bass_kernel_spmd(nc, in_maps, core_ids=list(range(B)))
    out = np.stack([res.results[c]["out0"] for c in range(B)], axis=1)
    return out.astype(np.float32)


if __name__ == "__main__":
    rng = np.random.default_rng(0)
    x = rng.standard_normal((S, B, D), np.float32)
    W = rng.standard_normal((NL, D, D), np.float32) / np.sqrt(D)
    b = rng.standard_normal((NL, D), np.float32) * 0.01
    out = kernel(x=x, W=W, b=b, alphas=np.ones(NL, np.float32), betas=np.zeros(NL, np.float32))
    print(out.shape, out.dtype)
